# revision 5
# baseline (speedup 1.0000x reference)
"""ALiBi causal attention on 8 TRN2 NeuronCores.

Sharding: batch (4) x head-group (2 groups of 8 heads) = 8 cores.
Per core: QKV projection for its 8 heads, banded causal attention
(ALiBi decay makes k < q-127 contribute exactly 0 in fp32, so only a
256-wide k-band per query is computed), pairwise AllGather of the
attention output, then each core of the pair computes half the output
rows of the out-projection. Matmuls run in float32r (full fp32 storage,
single-pass PE mode, ~1.5e-4 rel err).

Self-contained: only needs numpy/jax/concourse (available on the
grading container via the axon site packages).
"""
import numpy as np

B, S, D = 4, 2048, 1024
H, HD = 16, 64
NCORES = 8
HEADS_PER_CORE = 8
FLOC = HEADS_PER_CORE * HD  # 512 local features
NEG = np.float32(-1e30)

_CACHE = {}


def _build():
    import concourse.mybir as mybir
    import concourse.tile as tile
    from concourse import bacc

    F32 = mybir.dt.float32
    F32R = mybir.dt.float32r
    AF = mybir.ActivationFunctionType
    ADD = mybir.AluOpType.add
    MULT = mybir.AluOpType.mult

    nc = bacc.Bacc("TRN2", target_bir_lowering=False, debug=False, num_devices=NCORES)

    xT = nc.dram_tensor("xT", [D, S], F32, kind="ExternalInput").ap()
    wqkvT = nc.dram_tensor("wqkvT", [D, 3 * FLOC], F32, kind="ExternalInput").ap()
    woT = nc.dram_tensor("woT", [D, FLOC], F32, kind="ExternalInput").ap()
    maskc = nc.dram_tensor("maskc", [128, 256], F32, kind="ExternalInput").ap()
    onesc = nc.dram_tensor("onesc", [128, 128], F32, kind="ExternalInput").ap()
    bqk = nc.dram_tensor("bqk", [128, 8], F32, kind="ExternalInput").ap()
    bvrow = nc.dram_tensor("bvrow", [1, FLOC], F32, kind="ExternalInput").ap()
    bo = nc.dram_tensor("bo", [128, 4], F32, kind="ExternalInput").ap()
    out = nc.dram_tensor("out", [FLOC, S], F32, kind="ExternalOutput").ap()

    xT3 = xT.rearrange("(kt p) s -> p kt s", p=128)       # [128, 8, 2048]
    w3 = wqkvT.rearrange("(kt p) f -> p kt f", p=128)     # [128, 8, 1536]
    wo3 = woT.rearrange("(kt p) f -> p kt f", p=128)      # [128, 8, 512]

    NKT = S // 128   # 16 k-tiles
    NQT = S // 256   # 8 q-tiles

    from contextlib import ExitStack
    with tile.TileContext(nc) as tc:
        with (
            tc.tile_pool(name="const", bufs=1) as cpool,
            tc.tile_pool(name="dram", bufs=1, space="DRAM") as dram,
            ExitStack() as outer,
        ):
            stageAB = outer.enter_context(ExitStack())
            qkvpool = stageAB.enter_context(tc.tile_pool(name="qkv", bufs=1))
            stageA = ExitStack()
            wpool = stageA.enter_context(tc.tile_pool(name="w", bufs=1))
            xpool = stageA.enter_context(tc.tile_pool(name="xin", bufs=2))
            psA = stageA.enter_context(tc.tile_pool(name="psA", bufs=4, space="PSUM"))
            mask_sb = cpool.tile([128, 256], F32)
            ones_sb = cpool.tile([128, 128], F32R)
            bqk_sb = cpool.tile([128, 8], F32)
            bv_sb = cpool.tile([1, FLOC], F32R)
            bo_sb = cpool.tile([128, 4], F32)
            nc.sync.dma_start(mask_sb[:], maskc)
            nc.sync.dma_start(ones_sb[:], onesc.bitcast(F32R))
            nc.sync.dma_start(bqk_sb[:], bqk)
            nc.sync.dma_start(bv_sb[:], bvrow.bitcast(F32R))
            nc.sync.dma_start(bo_sb[:], bo)

            w_sb = wpool.tile([128, 8, 3 * FLOC], F32R)
            nc.sync.dma_start(w_sb[:], w3.bitcast(F32R))

            qt_sb = qkvpool.tile([128, 4, S], F32R, tag="q")
            kt_sb = qkvpool.tile([128, 4, S], F32R, tag="k")
            v_sb = qkvpool.tile([128, NKT, FLOC], F32R, tag="v")

            # ---- Stage A: projections, per s-chunk of 512 ----
            for sc in range(4):
                s0 = sc * 512
                xt = xpool.tile([128, 8, 512], F32R, tag="xt")
                nc.sync.dma_start(xt[:], xT3[:, :, s0 : s0 + 512].bitcast(F32R))
                # Q (mi 0..3) and K (mi 4..7): features on partitions
                for mi in range(8):
                    ps = psA.tile([128, 512], F32, tag="a")
                    for kt in range(8):
                        nc.tensor.matmul(
                            ps[:],
                            w_sb[:, kt, mi * 128 : mi * 128 + 128],
                            xt[:, kt, :],
                            start=(kt == 0),
                            stop=(kt == 7),
                        )
                    dst = qt_sb if mi < 4 else kt_sb
                    nc.scalar.activation(
                        dst[:, mi % 4, s0 : s0 + 512],
                        ps[:],
                        AF.Identity,
                        bias=bqk_sb[:, mi : mi + 1],
                    )
                # V (natural layout): s on partitions
                for si in range(4):
                    ps = psA.tile([128, 512], F32, tag="a")
                    nc.tensor.matmul(
                        ps[:], ones_sb[0:1, :], bv_sb[:], start=True, stop=False
                    )
                    for kt in range(8):
                        nc.tensor.matmul(
                            ps[:],
                            xt[:, kt, si * 128 : si * 128 + 128],
                            w_sb[:, kt, 2 * FLOC : 3 * FLOC],
                            start=False,
                            stop=(kt == 7),
                        )
                    nc.any.tensor_copy(v_sb[:, sc * 4 + si, :], ps[:])

            stageA.close()

            # ---- Stage B: attention per head ----
            # normalized attention pieces go straight to DRAM (ag_in)
            ag_in = dram.tile([FLOC, S], F32)
            spool = stageAB.enter_context(tc.tile_pool(name="small", bufs=4))
            ptpool = stageAB.enter_context(tc.tile_pool(name="pt", bufs=3))
            psS = stageAB.enter_context(tc.tile_pool(name="psS", bufs=2, space="PSUM"))
            psV = stageAB.enter_context(tc.tile_pool(name="psV", bufs=2, space="PSUM"))
            for h in range(HEADS_PER_CORE):
                mi_h, po = h // 2, (h % 2) * 64
                KTh = kt_sb[po : po + 64, mi_h, :]
                QTh = qt_sb[po : po + 64, mi_h, :]
                Vh = lambda kt: v_sb[:, kt, h * HD : (h + 1) * HD]

                # scores + exp per k-group of 4 k-tiles
                pts = []
                for G in range(4):
                    scps = psS.tile([128, 1024], F32, tag="sc")
                    pt = ptpool.tile([128, 4, 256], F32R, tag="pt")
                    ngrp = 4 if G < 3 else 3  # tile 15 is 128 wide
                    for j in range(4):
                        kt = 4 * G + j
                        k0 = kt * 128
                        qn = min(256, S - k0)
                        nc.tensor.matmul(
                            scps[:, j * 256 : j * 256 + qn],
                            KTh[:, k0 : k0 + 128],
                            QTh[:, k0 : k0 + qn],
                            start=True,
                            stop=True,
                        )
                    wid = ngrp * 256
                    sc3 = scps[:, 0:wid].rearrange("p (g f) -> p g f", f=256)
                    nc.vector.tensor_tensor(
                        sc3, sc3, mask_sb[:, None, :].to_broadcast((128, ngrp, 256)), ADD
                    )
                    nc.scalar.activation(
                        pt[:, 0:ngrp, :].rearrange("p g f -> p (g f)"),
                        scps[:, 0:wid],
                        AF.Exp,
                    )
                    if G == 3:
                        # last k-tile: 128 valid columns
                        nc.vector.tensor_tensor(
                            scps[:, 768:896], scps[:, 768:896],
                            mask_sb[:, 0:128], ADD,
                        )
                        nc.scalar.activation(pt[:, 3, 0:128], scps[:, 768:896], AF.Exp)
                    pts.append(pt)

                # PV + denominators per q-tile of 256
                for qt in range(NQT):
                    q0 = qt * 256
                    pv = psV.tile([64, 256], F32, tag="pv")
                    den = psV.tile([64, 256], F32, tag="den")
                    ktB = 2 * qt      # full [0:256]
                    ktA = 2 * qt - 1  # cols 128:256 -> pv[0:128]
                    ktC = 2 * qt + 1  # cols 0:128 -> pv[128:256]
                    rhsB = pts[ktB // 4][:, ktB % 4, :]
                    nc.tensor.matmul(pv[:], Vh(ktB), rhsB, start=True, stop=False)
                    nc.tensor.matmul(den[:], ones_sb[:, 0:64], rhsB, start=True, stop=False)
                    if ktA >= 0:
                        rhsA = pts[ktA // 4][:, ktA % 4, 128:256]
                        nc.tensor.matmul(pv[:, 0:128], Vh(ktA), rhsA, start=False, stop=False)
                        nc.tensor.matmul(den[:, 0:128], ones_sb[:, 0:64], rhsA, start=False, stop=False)
                    if ktC < NKT:
                        rhsC = pts[ktC // 4][:, ktC % 4, 0:128]
                        nc.tensor.matmul(pv[:, 128:256], Vh(ktC), rhsC, start=False, stop=True)
                        nc.tensor.matmul(den[:, 128:256], ones_sb[:, 0:64], rhsC, start=False, stop=True)
                    rec = spool.tile([64, 256], F32R, tag="rec")
                    with nc.allow_low_precision(reason="f32r rounding only"):
                        nc.vector.reciprocal(rec[:], den[:])
                    anorm = spool.tile([64, 256], F32, tag="anorm")
                    nc.vector.tensor_tensor(anorm[:], pv[:], rec[:], MULT)
                    r0 = mi_h * 128 + po
                    nc.sync.dma_start(ag_in[r0 : r0 + 64, q0 : q0 + 256], anorm[:])

            stageAB.close()

            # ---- Stage C: pairwise AllGather + half out-projection ----
            stageC = outer.enter_context(ExitStack())
            opool = stageC.enter_context(tc.tile_pool(name="oproj", bufs=2))
            wopool = stageC.enter_context(tc.tile_pool(name="wo", bufs=1))
            ytpool = stageC.enter_context(tc.tile_pool(name="yt", bufs=3))
            psC = stageC.enter_context(tc.tile_pool(name="psC", bufs=2, space="PSUM"))
            ag_out = dram.tile([2 * FLOC, S], F32)
            nc.gpsimd.collective_compute(
                "AllGather",
                mybir.AluOpType.bypass,
                replica_groups=[[0, 1], [2, 3], [4, 5], [6, 7]],
                ins=[ag_in[:].opt()],
                outs=[ag_out[:].opt()],
            )
            ag3 = ag_out[:].rearrange("(kt p) s -> p kt s", p=128)  # [128, 8, 2048]
            wo_sb = wopool.tile([128, 8, FLOC], F32R, tag="wo")
            nc.sync.dma_start(wo_sb[:], wo3.bitcast(F32R))
            for sb in range(4):
                s0 = sb * 512
                at = opool.tile([128, 8, 512], F32R, tag="at")
                nc.sync.dma_start(at[:], ag3[:, :, s0 : s0 + 512].bitcast(F32R))
                for mi in range(4):
                    ps = psC.tile([128, 512], F32, tag="c")
                    for kt in range(8):
                        nc.tensor.matmul(
                            ps[:],
                            wo_sb[:, kt, mi * 128 : mi * 128 + 128],
                            at[:, kt, :],
                            start=(kt == 0),
                            stop=(kt == 7),
                        )
                    yt = ytpool.tile([128, 512], F32, tag="yt")
                    nc.scalar.activation(
                        yt[:], ps[:], AF.Identity, bias=bo_sb[:, mi : mi + 1]
                    )
                    nc.sync.dma_start(
                        out[mi * 128 : mi * 128 + 128, s0 : s0 + 512], yt[:]
                    )
    nc.compile()
    return nc


def _get_runner():
    if "runner" in _CACHE:
        return _CACHE["runner"]
    import jax
    import numpy as _np
    from jax.sharding import Mesh, PartitionSpec, NamedSharding
    from jax.experimental.shard_map import shard_map
    import concourse.mybir as mybir
    from concourse.bass2jax import (
        _bass_exec_p,
        install_neuronx_cc_hook,
        partition_id_tensor,
    )

    nc = _build()
    install_neuronx_cc_hook()
    partition_name = nc.partition_id_tensor.name if nc.partition_id_tensor else None

    in_names, out_names, out_avals, zero_outs = [], [], [], []
    for alloc in nc.m.functions[0].allocations:
        if not isinstance(alloc, mybir.MemoryLocationSet):
            continue
        name = alloc.memorylocations[0].name
        if alloc.kind == "ExternalInput":
            if name != partition_name:
                in_names.append(name)
        elif alloc.kind == "ExternalOutput":
            shape = tuple(alloc.tensor_shape)
            dtype = mybir.dt.np(alloc.dtype)
            out_names.append(name)
            out_avals.append(jax.core.ShapedArray(shape, dtype))
            zero_outs.append(_np.zeros(shape, dtype))

    n_params = len(in_names)
    all_in_names = list(in_names) + list(out_names)
    if partition_name is not None:
        all_in_names.append(partition_name)

    def _body(*args):
        operands = list(args)
        if partition_name is not None:
            operands.append(partition_id_tensor())
        outs = _bass_exec_p.bind(
            *operands,
            out_avals=tuple(out_avals),
            in_names=tuple(all_in_names),
            out_names=tuple(out_names),
            lowering_input_output_aliases=(),
            sim_require_finite=True,
            sim_require_nnan=True,
            nc=nc,
        )
        return tuple(outs)

    devices = jax.devices()[:NCORES]
    mesh = Mesh(np.asarray(devices), ("core",))
    in_specs = (PartitionSpec("core"),) * (n_params + len(out_names))
    out_specs = (PartitionSpec("core"),) * len(out_names)
    fn = jax.jit(
        shard_map(_body, mesh=mesh, in_specs=in_specs, out_specs=out_specs,
                  check_rep=False),
        keep_unused=True,
    )
    sharding = NamedSharding(mesh, PartitionSpec("core"))
    runner = {
        "fn": fn,
        "in_names": in_names,
        "out_names": out_names,
        "out_avals": out_avals,
        "zero_outs": zero_outs,
        "sharding": sharding,
    }
    _CACHE["runner"] = runner
    return runner


def _prep_inputs(x, w_qkv, b_qkv, w_out, b_out):
    """Shard + lay out host-side. Returns list of per-core dicts."""
    x = np.asarray(x, np.float32)
    w_qkv = np.asarray(w_qkv, np.float32)
    b_qkv = np.asarray(b_qkv, np.float32)
    w_out = np.asarray(w_out, np.float32)
    b_out = np.asarray(b_out, np.float32)

    p_ = np.arange(128)[:, None]
    f_ = np.arange(256)[None, :]
    maskc = np.where(f_ >= p_, (p_ - f_).astype(np.float32), NEG)
    onesc = np.ones((128, 128), np.float32)

    scale = np.float32(1.0 / np.sqrt(HD))
    in_maps = []
    for c in range(NCORES):
        b, g = c // 2, c % 2
        fsl = slice(g * FLOC, (g + 1) * FLOC)
        wq = w_qkv[0 * D :][fsl, :] * scale
        wk = w_qkv[1 * D : 2 * D][fsl, :]
        wv = w_qkv[2 * D : 3 * D][fsl, :]
        bq = b_qkv[0 * D :][fsl] * scale
        bk = b_qkv[1 * D : 2 * D][fsl]
        bv = b_qkv[2 * D : 3 * D][fsl]
        osl = slice((c % 2) * FLOC, (c % 2 + 1) * FLOC)
        in_maps.append(
            {
                "xT": np.ascontiguousarray(x[b].T),
                "wqkvT": np.ascontiguousarray(
                    np.concatenate([wq, wk, wv], axis=0).T
                ),
                "woT": np.ascontiguousarray(w_out[osl, :].T),
                "maskc": maskc,
                "onesc": onesc,
                "bqk": np.ascontiguousarray(
                    np.concatenate([bq, bk]).reshape(8, 128).T
                ),
                "bvrow": bv.reshape(1, FLOC),
                "bo": np.ascontiguousarray(b_out[osl].reshape(4, 128).T),
            }
        )
    return in_maps


def _run_device(in_maps):
    import jax

    r = _get_runner()
    n = NCORES
    concat_in = [
        np.concatenate([np.asarray(in_maps[c][name]) for c in range(n)], axis=0)
        for name in r["in_names"]
    ]
    concat_zero = [
        np.zeros((n * z.shape[0], *z.shape[1:]), z.dtype) for z in r["zero_outs"]
    ]
    args = [jax.device_put(a, r["sharding"]) for a in concat_in + concat_zero]
    outs = r["fn"](*args)
    jax.block_until_ready(outs)
    oname = r["out_names"].index("out")
    full = np.asarray(outs[oname]).reshape(n, FLOC, S)
    return full, args


def kernel(x, w_qkv, b_qkv, w_out, b_out):
    in_maps = _prep_inputs(x, w_qkv, b_qkv, w_out, b_out)
    full, _ = _run_device(in_maps)
    # core 2b has y^T rows 0:512, core 2b+1 rows 512:1024 for batch b
    y = np.empty((B, S, D), np.float32)
    for b in range(B):
        yt = np.concatenate([full[2 * b], full[2 * b + 1]], axis=0)  # [1024, 2048]
        y[b] = yt.T
    return y
